# revision 1
# baseline (speedup 1.0000x reference)
"""HashEncoding (multires hash grid, 16 levels, F=2) for 8 trn2 NeuronCores.

Strategy (data-parallel per sharding hint): shard the 2^20 points across the
8 cores (131072 points/core), replicate the 67MB hash table. Per core the
Bass kernel computes, for each (point, level), the 8 corner hashes on the
Vector engine (integer ops; primes reduced mod 2^19 since only the low 19
hash bits survive the mask), gathers the 8 corner feature rows from the HBM
table via GPSIMD indirect DMA (one index per partition per instruction —
the only reliable form of that instruction), and blends trilinearly on the
Vector engine, accumulating all 16 levels in SBUF before one contiguous
output DMA per point-chunk.
"""
import sys

sys.path.insert(0, "/opt/trn_rl_repo")

import numpy as np

NUM_LEVELS = 16
F = 2
TABLE_SIZE = 1 << 19
MASK = TABLE_SIZE - 1
N_POINTS = 1 << 20
N_CORES = 8
PPC = 1024          # points per partition (131072 per core / 128)
CH = 64             # points per partition per chunk
SCALINGS = [16.0, 23.0, 33.0, 48.0, 70.0, 101.0, 147.0, 212.0,
            307.0, 445.0, 645.0, 933.0, 1351.0, 1955.0, 2830.0, 4095.0]
P1_19 = 489905      # 2654435761 mod 2^19
P2_19 = 153493      # 805459861  mod 2^19

_CACHE = {}


def build_program(ppc=PPC, ch=CH):
    import concourse.bass as bass
    import concourse.tile as tile
    from concourse import bacc, mybir

    F32 = mybir.dt.float32
    I32 = mybir.dt.int32
    npts = 128 * ppc
    nch = ppc // ch

    nc = bacc.Bacc("TRN2", target_bir_lowering=False, debug=False,
                   num_devices=N_CORES)
    xs = nc.dram_tensor("xs", [npts, 3], F32, kind="ExternalInput").ap()
    tbl = nc.dram_tensor("tbl", [TABLE_SIZE * NUM_LEVELS, F], F32,
                         kind="ExternalInput").ap()
    enc = nc.dram_tensor("enc", [npts, NUM_LEVELS * F], F32,
                         kind="ExternalOutput").ap()

    xr = xs.rearrange("(p k) d -> p (k d)", p=128)          # [128, ppc*3]
    enc_r = enc.rearrange("(p k) f -> p k f", p=128)        # [128, ppc, 32]

    XOR = mybir.AluOpType.bitwise_xor
    AND = mybir.AluOpType.bitwise_and
    OR = mybir.AluOpType.bitwise_or
    MUL = mybir.AluOpType.mult
    NE = mybir.AluOpType.not_equal
    GT = mybir.AluOpType.is_gt

    with tile.TileContext(nc) as tc:
        with (
            tc.tile_pool(name="xpool", bufs=1) as xpool,
            tc.tile_pool(name="scal", bufs=3) as scal,
            tc.tile_pool(name="gath", bufs=3) as gath,
            tc.tile_pool(name="accp", bufs=2) as accp,
        ):
            xt = xpool.tile([128, ppc * 3], F32)
            nc.sync.dma_start(xt[:], xr)
            x3 = xt.rearrange("p (k d) -> p k d", d=3)
            xd = []
            for d in range(3):
                t = xpool.tile([128, ppc], F32, tag=f"xd{d}")
                nc.vector.tensor_copy(t[:], x3[:, :, d])
                xd.append(t)

            with tc.For_i(0, nch) as i:
                acc = accp.tile([128, ch, NUM_LEVELS * F], F32, tag="acc")
                for lvl in range(NUM_LEVELS):
                    S = SCALINGS[lvl]
                    OFF = lvl << 19
                    sc, fl_f, off, fl_i, ce_i = [], [], [], [], []
                    for d in range(3):
                        s = scal.tile([128, ch], F32, tag=f"sc{d}")
                        nc.vector.tensor_scalar(
                            s[:], xd[d][:, bass.ts(i, ch)], S, None, MUL)
                        ri = scal.tile([128, ch], I32, tag=f"ri{d}")
                        nc.vector.tensor_copy(ri[:], s[:])     # round-nearest
                        rf = scal.tile([128, ch], F32, tag=f"rf{d}")
                        nc.vector.tensor_copy(rf[:], ri[:])
                        gt = scal.tile([128, ch], F32, tag=f"gt{d}")
                        nc.vector.tensor_tensor(gt[:], rf[:], s[:], GT)
                        ff = scal.tile([128, ch], F32, tag=f"ff{d}")
                        nc.vector.tensor_sub(ff[:], rf[:], gt[:])  # floor
                        od = scal.tile([128, ch], F32, tag=f"od{d}")
                        nc.vector.tensor_sub(od[:], s[:], ff[:])
                        ne = scal.tile([128, ch], F32, tag=f"ne{d}")
                        nc.vector.tensor_tensor(ne[:], ff[:], s[:], NE)
                        cf = scal.tile([128, ch], F32, tag=f"cf{d}")
                        nc.vector.tensor_add(cf[:], ff[:], ne[:])  # ceil
                        fi = scal.tile([128, ch], I32, tag=f"fi{d}")
                        nc.vector.tensor_copy(fi[:], ff[:])
                        ci = scal.tile([128, ch], I32, tag=f"ci{d}")
                        nc.vector.tensor_copy(ci[:], cf[:])
                        sc.append(s); fl_f.append(ff); off.append(od)
                        fl_i.append(fi); ce_i.append(ci)

                    # y*prime mod 2^19, split so every product is < 2^24
                    # (the DVE int multiply is fp32-backed: exact only there).
                    SHR = mybir.AluOpType.logical_shift_right
                    ADD = mybir.AluOpType.add

                    def hmul(src, prime, tag):
                        q1 = (prime * 32) & MASK
                        q2 = (prime * 1024) & MASK
                        y0 = scal.tile([128, ch], I32, tag=tag + "y0")
                        nc.vector.tensor_scalar(y0[:], src[:], 31, None, AND)
                        y1 = scal.tile([128, ch], I32, tag=tag + "y1")
                        nc.vector.tensor_scalar(y1[:], src[:], 5, 31, SHR, AND)
                        y2 = scal.tile([128, ch], I32, tag=tag + "y2")
                        nc.vector.tensor_scalar(y2[:], src[:], 10, None, SHR)
                        m0 = scal.tile([128, ch], I32, tag=tag + "m0")
                        nc.vector.tensor_scalar(m0[:], y0[:], prime, None, MUL)
                        nc.vector.tensor_scalar(m0[:], m0[:], MASK, None, AND)
                        m1 = scal.tile([128, ch], I32, tag=tag + "m1")
                        nc.vector.tensor_scalar(m1[:], y1[:], q1, None, MUL)
                        nc.vector.tensor_scalar(m1[:], m1[:], MASK, None, AND)
                        m2 = scal.tile([128, ch], I32, tag=tag + "m2")
                        nc.vector.tensor_scalar(m2[:], y2[:], q2, None, MUL)
                        h = scal.tile([128, ch], I32, tag=tag)
                        nc.vector.tensor_tensor(h[:], m0[:], m1[:], ADD)
                        nc.vector.tensor_tensor(h[:], h[:], m2[:], ADD)
                        return h

                    h1f = hmul(fl_i[1], P1_19, "h1f")
                    h1c = hmul(ce_i[1], P1_19, "h1c")
                    h2f = hmul(fl_i[2], P2_19, "h2f")
                    h2c = hmul(ce_i[2], P2_19, "h2c")

                    def txor(a, b, tag):
                        t = scal.tile([128, ch], I32, tag=tag)
                        nc.vector.tensor_tensor(t[:], a[:], b[:], XOR)
                        return t

                    t_cc = txor(h1c, h2c, "tcc")
                    t_fc = txor(h1f, h2c, "tfc")
                    t_cf = txor(h1c, h2f, "tcf")
                    t_ff = txor(h1f, h2f, "tff")

                    # corner order per reference CORNER_MASK (1=ceil):
                    # c0=(1,1,1) c1=(1,0,1) c2=(0,0,1) c3=(0,1,1)
                    # c4=(1,1,0) c5=(1,0,0) c6=(0,0,0) c7=(0,1,0)
                    xc, xf = ce_i[0], fl_i[0]
                    corner_spec = [(xc, t_cc), (xc, t_fc), (xf, t_fc),
                                   (xf, t_cc), (xc, t_cf), (xc, t_ff),
                                   (xf, t_ff), (xf, t_cf)]
                    idxs = []
                    for c, (xp, tp) in enumerate(corner_spec):
                        raw = scal.tile([128, ch], I32, tag=f"raw{c}")
                        nc.vector.tensor_tensor(raw[:], xp[:], tp[:], XOR)
                        ix = scal.tile([128, ch], I32, tag=f"ix{c}")
                        nc.vector.tensor_scalar(ix[:], raw[:], MASK, OFF,
                                                AND, OR)
                        idxs.append(ix)

                    # weights
                    ox, oy, oz = off
                    ws = []

                    def onem(o, tag):
                        t = scal.tile([128, ch], F32, tag=tag)
                        nc.vector.tensor_scalar(
                            t[:], o[:], -1.0, 1.0, MUL, mybir.AluOpType.add)
                        return t

                    bxx = onem(ox, "bxx")
                    byy = onem(oy, "byy")
                    bzz = onem(oz, "bzz")

                    def tmul(a, b, tag):
                        t = scal.tile([128, ch], F32, tag=tag)
                        nc.vector.tensor_mul(t[:], a[:], b[:])
                        return t

                    u_cc = tmul(oy, oz, "ucc")
                    u_fc = tmul(byy, oz, "ufc")
                    u_cf = tmul(oy, bzz, "ucf")
                    u_ff = tmul(byy, bzz, "uff")
                    wspec = [(ox, u_cc), (ox, u_fc), (bxx, u_fc), (bxx, u_cc),
                             (ox, u_cf), (ox, u_ff), (bxx, u_ff), (bxx, u_cf)]
                    for c, (a, b) in enumerate(wspec):
                        ws.append(tmul(a, b, f"w{c}"))

                    accsl = acc[:, :, lvl * F:(lvl + 1) * F]
                    for c in range(8):
                        g = gath.tile([128, ch * F], F32, tag=f"g{c}")
                        for k in range(ch):
                            nc.gpsimd.indirect_dma_start(
                                out=g[:, F * k:F * (k + 1)],
                                out_offset=None,
                                in_=tbl[:],
                                in_offset=bass.IndirectOffsetOnAxis(
                                    ap=idxs[c][:, k:k + 1], axis=0),
                            )
                        g3 = g.rearrange("p (k f) -> p k f", f=F)
                        wb = ws[c][:].to_broadcast([128, ch, F])
                        if c == 0:
                            nc.vector.tensor_tensor(accsl, g3[:], wb, MUL)
                        else:
                            pr = scal.tile([128, ch, F], F32, tag="pr")
                            nc.vector.tensor_tensor(pr[:], g3[:], wb, MUL)
                            nc.vector.tensor_add(accsl, accsl, pr[:])

                nc.sync.dma_start(enc_r[:, bass.ts(i, ch), :], acc[:])
    nc.compile()
    return nc


def _get_program():
    key = (PPC, CH)
    if key not in _CACHE:
        _CACHE[key] = build_program()
    return _CACHE[key]


def kernel(x: np.ndarray, hash_table: np.ndarray) -> np.ndarray:
    from concourse.bass_utils import run_bass_kernel_spmd

    nc = _get_program()
    x = np.ascontiguousarray(np.asarray(x, dtype=np.float32))
    tb = np.ascontiguousarray(np.asarray(hash_table, dtype=np.float32))
    npc = N_POINTS // N_CORES
    in_maps = [
        {"xs": x[c * npc:(c + 1) * npc], "tbl": tb} for c in range(N_CORES)
    ]
    res = run_bass_kernel_spmd(nc, in_maps, list(range(N_CORES)))
    return np.concatenate(
        [res.results[c]["enc"] for c in range(N_CORES)], axis=0)

